# revision 14
# baseline (speedup 1.0000x reference)
"""Bahdanau pointer-attention kernel for Trainium2 (8 NeuronCores, SPMD).

Computes energy[b, 1, n] = V . tanh(x[b, :N] @ W1.T + x[b, -1] @ W2.T)
for B=32, N=2048, D=1024.

Sharding: data-parallel over batch B across 8 cores (4 batches/core).
Per-core layout: contraction over d requires d on SBUF partitions, so the
host pre-transposes each core's x shard to [D, 4*N] during sharding.

Per-core pipeline (Tile framework), all matmul operands bf16:
  - keys matmul (PE): psum[e128, n512] += W1T[d128, e128].T @ xT[d128, n512]
  - ACT: tanh(psum + query_bias) fused via activation bias (per-partition)
  - DVE: acc[e128, n512] (+)= V[e] * tanh_tile   (scalar_tensor_tensor)
  - Pool: en[1, n512] = reduce_C(acc)            (partition reduction)
  so the PE runs ONLY the keys matmuls (plus a tiny query preamble) and
  stays continuously busy -> full 2.4 GHz p-state.
  - query preamble: psum[e128, b4] += W2T[d128, e128].T @ xqT[d128, b4]

Weight DMAs are split per-ec chunk so the first keys matmul can start
~1us after launch instead of waiting for the full 4MB of weights.
"""

from contextlib import ExitStack

import numpy as np
import ml_dtypes

import concourse.bass as bass
import concourse.mybir as mybir
import concourse.tile as tile
from concourse import bacc
from concourse.bass_utils import run_bass_kernel_spmd

B, N, D = 32, 2048, 1024
CORES = 8
BPC = B // CORES            # batches per core
NTOT = BPC * N              # 8192 key positions per core
P = 128
DC = D // P                 # 8 d-chunks (contraction)
EC = D // P                 # 8 e-chunks (output feature)
NT = 512                    # n tile (one PSUM bank of f32)
NCH = NTOT // NT            # 16 n-chunks per core
NPB = N // NT               # n-chunks per batch

f32 = mybir.dt.float32
f32r = mybir.dt.float32r
bf16 = mybir.dt.bfloat16

TRACE = False
LAST_EXEC_NS = None
LAST_RESULTS = None

_NC_CACHE = {}


def _body(ctx, tc, xT, xqT, w1T, w2T, vT, out, reps=1, mode="ones",
          kpsum_bufs=None, x_bufs=3, acc_dt=f32, xw_dt=bf16,
          do_dma=True, do_mm=True, do_act=True, do_dve=True, do_fin=True,
          split_ch0=True, query_at=3):
    nc = tc.nc
    Tanh = mybir.ActivationFunctionType.Tanh
    Mult = mybir.AluOpType.mult
    Add = mybir.AluOpType.add

    if kpsum_bufs is None:
        kpsum_bufs = 7 if mode == "gpsimd" else 6
    w_pool = ctx.enter_context(tc.tile_pool(name="w", bufs=1))
    x_pool = ctx.enter_context(tc.tile_pool(name="x", bufs=x_bufs))
    t_pool = ctx.enter_context(tc.tile_pool(name="tanh", bufs=2 * EC + 2))
    acc_pool = ctx.enter_context(tc.tile_pool(name="acc", bufs=3))
    small = ctx.enter_context(tc.tile_pool(name="small", bufs=1))
    en_pool = ctx.enter_context(tc.tile_pool(name="en", bufs=3))
    kpsum = ctx.enter_context(
        tc.tile_pool(name="kpsum", bufs=kpsum_bufs, space="PSUM"))
    vpsum = None
    if mode != "gpsimd":
        vpsum = ctx.enter_context(
            tc.tile_pool(name="vpsum", bufs=1, space="PSUM"))
    qpsum = ctx.enter_context(tc.tile_pool(name="qpsum", bufs=1, space="PSUM"))

    # Small operands first so they don't queue behind the weight DMAs.
    xq_sb = small.tile([P, DC, BPC], xw_dt, tag="xq")
    nc.sync.dma_start(xq_sb[:], xqT.rearrange("p (c b) -> p c b", b=BPC))
    v_sb = small.tile([P, EC], f32, tag="v")
    nc.sync.dma_start(v_sb[:], vT[:, :])
    ones = None
    if mode == "ones":
        acc_dt = bf16
        ones = small.tile([P, 1], bf16, tag="ones")
        nc.vector.memset(ones[:], 1.0)
    if mode == "pe":
        v_bf = small.tile([P, EC], bf16, tag="vbf")
        nc.vector.tensor_copy(v_bf[:], v_sb[:])
        v_sb = v_bf

    # Resident weights, per-ec DMA granularity: [p=128, ec, dc, e128]
    w1_sb = w_pool.tile([P, EC, DC, P], xw_dt, tag="w1")
    w2_sb = w_pool.tile([P, EC, DC, P], xw_dt, tag="w2")
    for ec in range(EC):
        nc.scalar.dma_start(
            w1_sb[:, ec], w1T[ec:ec + 1].rearrange("e p (c q) -> (e p) c q", q=P))
        nc.gpsimd.dma_start(
            w2_sb[:, ec], w2T[ec:ec + 1].rearrange("e p (c q) -> (e p) c q", q=P))

    # Query: q_sb[e128, (ec, b)] = x_query @ W2.T  (transposed). All 8 ec
    # groups accumulate into one PSUM bank at different free offsets; each
    # gets its own DVE copy so tanh(ec) unblocks early. Emitted into the PE
    # stream after `query_at` keys groups of chunk 0 so the PE does not sit
    # idle waiting for the W2 DMA at startup (that would also reset the
    # p-state ramp).
    q_sb = small.tile([P, EC * BPC], f32, tag="q")
    pq = qpsum.tile([P, EC * BPC], f32, tag="pq")

    def emit_query():
        for ec in range(EC):
            for dc in range(DC):
                nc.tensor.matmul(
                    pq[:, ec * BPC:(ec + 1) * BPC],
                    lhsT=w2_sb[:, ec, dc],
                    rhs=xq_sb[:, dc],
                    start=(dc == 0),
                    stop=(dc == DC - 1),
                )
            nc.vector.tensor_copy(q_sb[:, ec * BPC:(ec + 1) * BPC],
                                  pq[:, ec * BPC:(ec + 1) * BPC])

    if not do_mm:
        emit_query()

    xTr = xT.rearrange("(c p) n -> p c n", p=P)
    pend = None
    x_fixed = None
    if not do_dma:
        x_fixed = x_pool.tile([P, DC, NT], xw_dt, tag="x")
        nc.sync.dma_start(x_fixed[:], xTr[:, :, 0:NT])

    for it in range(reps * NCH):
        ch = it % NCH
        b = ch // NPB
        if do_dma:
            x_sb = x_pool.tile([P, DC, NT], xw_dt, tag="x")
            src = xTr[:, :, ch * NT:(ch + 1) * NT]
            if it == 0 and split_ch0:
                # Per-dc DMAs so the first matmul starts ~0.4us in.
                for dc in range(DC):
                    nc.sync.dma_start(x_sb[:, dc:dc + 1], src[:, dc:dc + 1])
            else:
                nc.sync.dma_start(x_sb[:], src)
        else:
            x_sb = x_fixed
        if not do_mm:
            continue
        acc = None
        if mode != "pe":
            acc = acc_pool.tile([P, NT], acc_dt, tag="acc", name="acc")
        tts = []
        for ec in range(EC):
            if it == 0 and ec == query_at:
                emit_query()
            pk = kpsum.tile([P, NT], f32, tag="pk")
            for dc in range(DC):
                nc.tensor.matmul(
                    pk[:],
                    lhsT=w1_sb[:, ec, dc],
                    rhs=x_sb[:, dc],
                    start=(dc == 0),
                    stop=(dc == DC - 1),
                )
            if not do_act:
                continue
            tt = t_pool.tile([P, NT], bf16, tag="tanh")
            nc.scalar.activation(
                tt[:], pk[:], Tanh,
                bias=q_sb[:, ec * BPC + b: ec * BPC + b + 1],
            )
            if mode == "pe":
                tts.append(tt)
                continue
            if not do_dve:
                continue
            if ec == 0:
                nc.vector.tensor_scalar_mul(acc[:], tt[:], v_sb[:, 0:1])
            else:
                nc.vector.scalar_tensor_tensor(
                    acc[:], tt[:], v_sb[:, ec:ec + 1], acc[:],
                    op0=Mult, op1=Add)
        if not do_act or not do_fin or (mode != "pe" and not do_dve):
            continue
        # Finalize chunk k-1 after the keys matmuls of chunk k are queued
        # so the PE / Pool never make the PE wait.
        if pend is not None:
            _finalize(nc, mode, vpsum, en_pool, ones, v_sb, out, *pend)
        pend = ((acc if mode != "pe" else tts), ch)
    if pend is not None:
        _finalize(nc, mode, vpsum, en_pool, ones, v_sb, out, *pend)


def _finalize(nc, mode, vpsum, en_pool, ones, v_sb, out, payload, ch):
    if mode == "gpsimd":
        acc = payload
        en = en_pool.tile([1, NT], f32, tag="en")
        nc.gpsimd.tensor_reduce(
            en[:], acc[:], axis=mybir.AxisListType.C, op=mybir.AluOpType.add)
        nc.sync.dma_start(out[ch:ch + 1, :], en[:])
        return
    if mode == "ones":
        acc = payload
        pv = vpsum.tile([1, NT], f32, tag="pv")
        nc.tensor.matmul(
            pv[:],
            lhsT=ones[:],
            rhs=acc[:],
            start=True, stop=True,
        )
        en = en_pool.tile([1, NT], f32, tag="en")
        nc.vector.tensor_copy(en[:], pv[:])
        nc.sync.dma_start(out[ch:ch + 1, :], en[:])
        return
    # mode == "pe": baseline-style V-dot on the tensor engine (bf16).
    tts = payload
    pv = vpsum.tile([1, NT], f32, tag="pv")
    for ec in range(EC):
        nc.tensor.matmul(
            pv[:],
            lhsT=v_sb[:, ec:ec + 1],
            rhs=tts[ec][:],
            start=(ec == 0),
            stop=(ec == EC - 1),
        )
    en = en_pool.tile([1, NT], f32, tag="en")
    nc.vector.tensor_copy(en[:], pv[:])
    nc.sync.dma_start(out[ch:ch + 1, :], en[:])


def build_module(reps=1, **opts):
    key = (reps, tuple(sorted(opts.items())))
    if key in _NC_CACHE:
        return _NC_CACHE[key]
    nc = bacc.Bacc("TRN2", target_bir_lowering=False, debug=False)
    xw_dt = opts.get("xw_dt", bf16)
    xT = nc.declare_dram_parameter("xT", [D, NTOT], xw_dt, isOutput=False)
    xqT = nc.declare_dram_parameter("xqT", [P, DC * BPC], xw_dt, isOutput=False)
    w1T = nc.declare_dram_parameter("w1T", [EC, P, DC * P], xw_dt, isOutput=False)
    w2T = nc.declare_dram_parameter("w2T", [EC, P, DC * P], xw_dt, isOutput=False)
    vT = nc.declare_dram_parameter("vT", [P, EC], f32, isOutput=False)
    out = nc.declare_dram_parameter("out", [NCH, NT], f32, isOutput=True)
    with tile.TileContext(nc) as tc:
        with ExitStack() as ctx:
            _body(ctx, tc, xT, xqT, w1T, w2T, vT, out, reps=reps, **opts)
    nc.compile()
    _NC_CACHE[key] = nc
    return nc


def shard_inputs(x, W1, W2, V, xw_dt="bf16"):
    """Host-side sharding + layout transforms. Returns per-core input maps."""
    x = np.asarray(x, dtype=np.float32)
    xdt = ml_dtypes.bfloat16 if xw_dt == "bf16" else np.float32

    def w_layout(W):
        # [ec, p, (dc q)] with W[e, d]: entry = W[ec*P + q, dc*P + p]
        Wr = np.asarray(W, np.float32).reshape(EC, P, DC, P)  # [ec, q, dc, p]
        return np.ascontiguousarray(
            Wr.transpose(0, 3, 2, 1).reshape(EC, P, DC * P)).astype(xdt)

    w1T = w_layout(W1)
    w2T = w_layout(W2)
    vT = np.ascontiguousarray(np.asarray(V, np.float32).reshape(EC, P).T)
    in_maps = []
    for c in range(CORES):
        xs = x[c * BPC:(c + 1) * BPC, :N, :]          # [BPC, N, D]
        xT = np.ascontiguousarray(xs.transpose(2, 0, 1)).reshape(D, NTOT).astype(xdt)
        xq = x[c * BPC:(c + 1) * BPC, N, :]           # [BPC, D]
        # [p, (dc b)]: entry = xq[b, dc*P + p]
        xqT = np.ascontiguousarray(
            xq.T.reshape(DC, P, BPC).transpose(1, 0, 2).reshape(P, DC * BPC)
        ).astype(xdt)
        in_maps.append({
            "xT": xT, "xqT": xqT,
            "w1T": w1T, "w2T": w2T, "vT": vT,
        })
    return in_maps


def kernel(x, W1, W2, V, city_count):
    global LAST_EXEC_NS, LAST_RESULTS
    assert int(city_count) == N
    nc = build_module()
    in_maps = shard_inputs(x, W1, W2, V)
    res = run_bass_kernel_spmd(nc, in_maps, core_ids=list(range(CORES)),
                               trace=TRACE)
    LAST_EXEC_NS = res.exec_time_ns
    LAST_RESULTS = res
    out = np.concatenate(
        [res.results[c]["out"].reshape(BPC, N) for c in range(CORES)], axis=0
    )
    return out[:, None, :].astype(np.float32)


# revision 16
# speedup vs baseline: 1.4232x; 1.4232x over previous
"""Bahdanau pointer-attention kernel for Trainium2 (8 NeuronCores, SPMD).

Computes energy[b, 1, n] = V . tanh(x[b, :N] @ W1.T + x[b, -1] @ W2.T)
for B=32, N=2048, D=1024.

Sharding: data-parallel over batch B across 8 cores (4 batches/core).
Per-core layout: contraction over d requires d on SBUF partitions, so the
host pre-transposes each core's x shard to [D, 4*N] during sharding.

Per-core pipeline (Tile framework), all matmul operands bf16:
  - keys matmul (PE): psum[e128, n512] += W1T[d128, e128].T @ xT[d128, n512]
  - ACT: tanh(psum + query_bias) fused via activation bias (per-partition)
  - DVE: acc[e128, n512] (+)= V[e] * tanh_tile   (scalar_tensor_tensor)
  - Pool: en[1, n512] = reduce_C(acc)            (partition reduction)
  so the PE runs ONLY the keys matmuls (plus a tiny query preamble) and
  stays continuously busy -> full 2.4 GHz p-state.
  - query preamble: psum[e128, b4] += W2T[d128, e128].T @ xqT[d128, b4]

Weight DMAs are split per-ec chunk so the first keys matmul can start
~1us after launch instead of waiting for the full 4MB of weights.
"""

from contextlib import ExitStack

import numpy as np
import ml_dtypes

import concourse.bass as bass
import concourse.mybir as mybir
import concourse.tile as tile
from concourse import bacc
from concourse.bass_utils import run_bass_kernel_spmd

B, N, D = 32, 2048, 1024
CORES = 8
BPC = B // CORES            # batches per core
NTOT = BPC * N              # 8192 key positions per core
P = 128
DC = D // P                 # 8 d-chunks (contraction)
EC = D // P                 # 8 e-chunks (output feature)
NT = 512                    # n tile (one PSUM bank of f32)
NCH = NTOT // NT            # 16 n-chunks per core
NPB = N // NT               # n-chunks per batch

f32 = mybir.dt.float32
f32r = mybir.dt.float32r
bf16 = mybir.dt.bfloat16

TRACE = False
LAST_EXEC_NS = None
LAST_RESULTS = None

_NC_CACHE = {}


def _body(ctx, tc, xT, xqT, w1T, w2T, vT, out, reps=1, mode="ones",
          kpsum_bufs=None, x_bufs=3, acc_dt=f32, xw_dt=bf16,
          do_dma=True, do_mm=True, do_act=True, do_dve=True, do_fin=True,
          split_ch0=True, query_at=3, same_w=False):
    nc = tc.nc
    Tanh = mybir.ActivationFunctionType.Tanh
    Mult = mybir.AluOpType.mult
    Add = mybir.AluOpType.add

    if kpsum_bufs is None:
        kpsum_bufs = 7 if mode == "gpsimd" else 6
    w_pool = ctx.enter_context(tc.tile_pool(name="w", bufs=1))
    x_pool = ctx.enter_context(tc.tile_pool(name="x", bufs=x_bufs))
    t_pool = ctx.enter_context(tc.tile_pool(name="tanh", bufs=2 * EC + 2))
    acc_pool = ctx.enter_context(tc.tile_pool(name="acc", bufs=3))
    small = ctx.enter_context(tc.tile_pool(name="small", bufs=1))
    en_pool = ctx.enter_context(tc.tile_pool(name="en", bufs=3))
    kpsum = ctx.enter_context(
        tc.tile_pool(name="kpsum", bufs=kpsum_bufs, space="PSUM"))
    vpsum = None
    if mode != "gpsimd":
        vpsum = ctx.enter_context(
            tc.tile_pool(name="vpsum", bufs=1, space="PSUM"))
    qpsum = ctx.enter_context(tc.tile_pool(name="qpsum", bufs=1, space="PSUM"))

    # Small operands first so they don't queue behind the weight DMAs.
    xq_sb = small.tile([P, DC, BPC], xw_dt, tag="xq")
    nc.sync.dma_start(xq_sb[:], xqT.rearrange("p (c b) -> p c b", b=BPC))
    v_sb = small.tile([P, EC], f32, tag="v")
    nc.sync.dma_start(v_sb[:], vT[:, :])
    ones = None
    if mode == "ones":
        acc_dt = bf16
        ones = small.tile([P, 1], bf16, tag="ones")
        nc.vector.memset(ones[:], 1.0)
    if mode == "pe":
        v_bf = small.tile([P, EC], bf16, tag="vbf")
        nc.vector.tensor_copy(v_bf[:], v_sb[:])
        v_sb = v_bf

    # Resident weights, per-ec DMA granularity: [p=128, ec, dc, e128]
    w1_sb = w_pool.tile([P, EC, DC, P], xw_dt, tag="w1")
    w2_sb = w_pool.tile([P, EC, DC, P], xw_dt, tag="w2")
    for ec in range(EC):
        nc.scalar.dma_start(
            w1_sb[:, ec], w1T[ec:ec + 1].rearrange("e p (c q) -> (e p) c q", q=P))
        nc.gpsimd.dma_start(
            w2_sb[:, ec], w2T[ec:ec + 1].rearrange("e p (c q) -> (e p) c q", q=P))

    # Query: q_sb[e128, (ec, b)] = x_query @ W2.T  (transposed). All 8 ec
    # groups accumulate into one PSUM bank at different free offsets; each
    # gets its own DVE copy so tanh(ec) unblocks early. Emitted into the PE
    # stream after `query_at` keys groups of chunk 0 so the PE does not sit
    # idle waiting for the W2 DMA at startup (that would also reset the
    # p-state ramp).
    q_sb = small.tile([P, EC * BPC], f32, tag="q")
    pq = qpsum.tile([P, EC * BPC], f32, tag="pq")

    def emit_query():
        for ec in range(EC):
            for dc in range(DC):
                nc.tensor.matmul(
                    pq[:, ec * BPC:(ec + 1) * BPC],
                    lhsT=w2_sb[:, ec, dc],
                    rhs=xq_sb[:, dc],
                    start=(dc == 0),
                    stop=(dc == DC - 1),
                )
            nc.vector.tensor_copy(q_sb[:, ec * BPC:(ec + 1) * BPC],
                                  pq[:, ec * BPC:(ec + 1) * BPC])

    if not do_mm:
        emit_query()

    xTr = xT.rearrange("(c p) n -> p c n", p=P)
    pend = None
    x_fixed = None
    if not do_dma:
        x_fixed = x_pool.tile([P, DC, NT], xw_dt, tag="x")
        nc.sync.dma_start(x_fixed[:], xTr[:, :, 0:NT])

    for it in range(reps * NCH):
        ch = it % NCH
        b = ch // NPB
        if do_dma:
            x_sb = x_pool.tile([P, DC, NT], xw_dt, tag="x")
            src = xTr[:, :, ch * NT:(ch + 1) * NT]
            if it == 0 and split_ch0:
                # Per-dc DMAs so the first matmul starts ~0.4us in.
                for dc in range(DC):
                    nc.sync.dma_start(x_sb[:, dc:dc + 1], src[:, dc:dc + 1])
            else:
                nc.sync.dma_start(x_sb[:], src)
        else:
            x_sb = x_fixed
        if not do_mm:
            continue
        acc = None
        if mode != "pe":
            acc = acc_pool.tile([P, NT], acc_dt, tag="acc", name="acc")
        tts = []
        for ec in range(EC):
            if it == 0 and ec == query_at:
                emit_query()
            pk = kpsum.tile([P, NT], f32, tag="pk")
            for dc in range(DC):
                nc.tensor.matmul(
                    pk[:],
                    lhsT=w1_sb[:, 0, 0] if same_w else w1_sb[:, ec, dc],
                    rhs=x_sb[:, dc],
                    start=(dc == 0),
                    stop=(dc == DC - 1),
                )
            if not do_act:
                continue
            tt = t_pool.tile([P, NT], bf16, tag="tanh")
            nc.scalar.activation(
                tt[:], pk[:], Tanh,
                bias=q_sb[:, ec * BPC + b: ec * BPC + b + 1],
            )
            if mode == "pe":
                tts.append(tt)
                continue
            if not do_dve:
                continue
            if ec == 0:
                nc.vector.tensor_scalar_mul(acc[:], tt[:], v_sb[:, 0:1])
            else:
                nc.vector.scalar_tensor_tensor(
                    acc[:], tt[:], v_sb[:, ec:ec + 1], acc[:],
                    op0=Mult, op1=Add)
        if not do_act or not do_fin or (mode != "pe" and not do_dve):
            continue
        # Finalize chunk k-1 after the keys matmuls of chunk k are queued
        # so the PE / Pool never make the PE wait.
        if pend is not None:
            _finalize(nc, mode, vpsum, en_pool, ones, v_sb, out, *pend)
        pend = ((acc if mode != "pe" else tts), ch)
    if pend is not None:
        _finalize(nc, mode, vpsum, en_pool, ones, v_sb, out, *pend)


def _finalize(nc, mode, vpsum, en_pool, ones, v_sb, out, payload, ch):
    if mode == "gpsimd":
        acc = payload
        en = en_pool.tile([1, NT], f32, tag="en")
        nc.gpsimd.tensor_reduce(
            en[:], acc[:], axis=mybir.AxisListType.C, op=mybir.AluOpType.add)
        nc.sync.dma_start(out[ch:ch + 1, :], en[:])
        return
    if mode == "ones":
        acc = payload
        pv = vpsum.tile([1, NT], f32, tag="pv")
        nc.tensor.matmul(
            pv[:],
            lhsT=ones[:],
            rhs=acc[:],
            start=True, stop=True,
        )
        en = en_pool.tile([1, NT], f32, tag="en")
        nc.vector.tensor_copy(en[:], pv[:])
        nc.sync.dma_start(out[ch:ch + 1, :], en[:])
        return
    # mode == "pe": baseline-style V-dot on the tensor engine (bf16).
    tts = payload
    pv = vpsum.tile([1, NT], f32, tag="pv")
    for ec in range(EC):
        nc.tensor.matmul(
            pv[:],
            lhsT=v_sb[:, ec:ec + 1],
            rhs=tts[ec][:],
            start=(ec == 0),
            stop=(ec == EC - 1),
        )
    en = en_pool.tile([1, NT], f32, tag="en")
    nc.vector.tensor_copy(en[:], pv[:])
    nc.sync.dma_start(out[ch:ch + 1, :], en[:])


def build_module(reps=1, **opts):
    key = (reps, tuple(sorted(opts.items())))
    if key in _NC_CACHE:
        return _NC_CACHE[key]
    nc = bacc.Bacc("TRN2", target_bir_lowering=False, debug=False)
    xw_dt = opts.get("xw_dt", bf16)
    xT = nc.declare_dram_parameter("xT", [D, NTOT], xw_dt, isOutput=False)
    xqT = nc.declare_dram_parameter("xqT", [P, DC * BPC], xw_dt, isOutput=False)
    w1T = nc.declare_dram_parameter("w1T", [EC, P, DC * P], xw_dt, isOutput=False)
    w2T = nc.declare_dram_parameter("w2T", [EC, P, DC * P], xw_dt, isOutput=False)
    vT = nc.declare_dram_parameter("vT", [P, EC], f32, isOutput=False)
    out = nc.declare_dram_parameter("out", [NCH, NT], f32, isOutput=True)
    with tile.TileContext(nc) as tc:
        with ExitStack() as ctx:
            _body(ctx, tc, xT, xqT, w1T, w2T, vT, out, reps=reps, **opts)
    nc.compile()
    _NC_CACHE[key] = nc
    return nc


def shard_inputs(x, W1, W2, V, xw_dt="bf16"):
    """Host-side sharding + layout transforms. Returns per-core input maps."""
    x = np.asarray(x, dtype=np.float32)
    xdt = ml_dtypes.bfloat16 if xw_dt == "bf16" else np.float32

    def w_layout(W):
        # [ec, p, (dc q)] with W[e, d]: entry = W[ec*P + q, dc*P + p]
        Wr = np.asarray(W, np.float32).reshape(EC, P, DC, P)  # [ec, q, dc, p]
        return np.ascontiguousarray(
            Wr.transpose(0, 3, 2, 1).reshape(EC, P, DC * P)).astype(xdt)

    w1T = w_layout(W1)
    w2T = w_layout(W2)
    vT = np.ascontiguousarray(np.asarray(V, np.float32).reshape(EC, P).T)
    in_maps = []
    for c in range(CORES):
        xs = x[c * BPC:(c + 1) * BPC, :N, :]          # [BPC, N, D]
        xT = np.ascontiguousarray(xs.transpose(2, 0, 1)).reshape(D, NTOT).astype(xdt)
        xq = x[c * BPC:(c + 1) * BPC, N, :]           # [BPC, D]
        # [p, (dc b)]: entry = xq[b, dc*P + p]
        xqT = np.ascontiguousarray(
            xq.T.reshape(DC, P, BPC).transpose(1, 0, 2).reshape(P, DC * BPC)
        ).astype(xdt)
        in_maps.append({
            "xT": xT, "xqT": xqT,
            "w1T": w1T, "w2T": w2T, "vT": vT,
        })
    return in_maps


def kernel(x, W1, W2, V, city_count):
    global LAST_EXEC_NS, LAST_RESULTS
    assert int(city_count) == N
    nc = build_module()
    in_maps = shard_inputs(x, W1, W2, V)
    res = run_bass_kernel_spmd(nc, in_maps, core_ids=list(range(CORES)),
                               trace=TRACE)
    LAST_EXEC_NS = res.exec_time_ns
    LAST_RESULTS = res
    out = np.concatenate(
        [res.results[c]["out"].reshape(BPC, N) for c in range(CORES)], axis=0
    )
    return out[:, None, :].astype(np.float32)
